# revision 1
# baseline (speedup 1.0000x reference)
"""Trainium2 Bass kernel for nn_BDHAttention (RoPE(Q) self-score attention, no softmax).

Per (batch, head) slice s: QR = rope(Q_s) [T,N]; S = QR @ QR.T / sqrt(N) [T,T];
O_s = S @ V_s [T,N].  K input is unused by the reference.  B*nh = 8 slices map
1:1 onto the 8 NeuronCores (data/head parallel, no communication).

Device-side structure per core (T=2048, N=4096, P=128):
  - Q arrives fp16 with its feature dim de-interleaved on the host
    ([evens | odds]) so RoPE is all contiguous 16-bit tensor_tensor ops
    (DVE 2x mode).  The n-permutation is harmless: it is the contraction
    dim of S = QR @ QR.T and both operands share it.
  - cos/sin tables are host-precomputed fp16, scaled by 1/8 each so S picks
    up the 1/64 = 1/sqrt(N) scale for free.
  - PE-transpose QR' 128x128 tiles into two resident fp16 panels
    (QR'^T, t-halves A and B).  Panel-B build is interleaved with the
    S[A,A] matmuls to keep the PE dense (HAM stays warm; junk identity
    matmuls fill the DVE-bound build windows).
  - MM1 (fp16, fp32 PSUM accum): S[A,A], S[B,B], S[A,B] all from resident
    panels.  Within the diagonal quadrants only on/above-diagonal 128-blocks
    are computed; below-diagonal blocks (and all of S[B,A]) are filled by
    PE-transposing the computed mirrors (S is symmetric).  S stored fp16 in
    a DRAM scratch.
  - MM2: O = S @ V.  S row-panels re-read from DRAM serve directly as lhsT
    tiles (partition = contraction dim) thanks to S's symmetry; V streamed
    fp16; O accumulated fp32 in PSUM and written out fp32.
"""

import math
import sys

sys.path.insert(0, "/opt/trn_rl_repo")

import numpy as np

import concourse.bacc as bacc
import concourse.mybir as mybir
import concourse.tile as tile
from concourse.bass_utils import run_bass_kernel_spmd

B, NH, T, N = 2, 4, 2048, 4096
THETA = 2 ** 16
P = 128
HALF = T // 2            # 1024
NTILES = T // P          # 16 t-tiles
NCH = N // P             # 32 n-chunks
F = 512                  # matmul moving free dim (one fp32 PSUM bank)
H = N // 2               # 2048

f16 = mybir.dt.float16
f32 = mybir.dt.float32


def _build_nc():
    nc = bacc.Bacc("TRN2", target_bir_lowering=False, debug=False, num_devices=8)

    q = nc.dram_tensor("q", [T, N], f16, kind="ExternalInput")
    v = nc.dram_tensor("v", [T, N], f16, kind="ExternalInput")
    cu = nc.dram_tensor("cu", [T, H], f16, kind="ExternalInput")
    su = nc.dram_tensor("su", [T, H], f16, kind="ExternalInput")
    ident = nc.dram_tensor("ident", [P, P], f16, kind="ExternalInput")
    o = nc.dram_tensor("o", [T, N], f32, kind="ExternalOutput")

    with tile.TileContext(nc) as tc:
        with (
            tc.tile_pool(name="dram", bufs=1, space="DRAM") as dram,
            tc.tile_pool(name="const", bufs=1) as const,
            tc.tile_pool(name="panel", bufs=1) as panel,
            tc.tile_pool(name="ps", bufs=1, space="PSUM") as ps,
            tc.tile_pool(name="work", bufs=1) as work,
        ):
            s_mat = dram.tile([T, T], f16, name="s_mat")

            idt = const.tile([P, P], f16, name="idt")
            nc.sync.dma_start(idt[:], ident.ap())

            pa = [
                panel.tile([P, HALF], f16, name=f"pk_a{k}", tag=f"pk_a{k}")
                for k in range(NCH)
            ]
            pb = [
                panel.tile([P, HALF], f16, name=f"pk_b{k}", tag=f"pk_b{k}")
                for k in range(NCH)
            ]

            def build_tile(dst, half, ti):
                """RoPE t-tile (half*8 + ti) and transpose its 32 n-chunks into
                panel columns ti*P:(ti+1)*P."""
                trow = half * (NTILES // 2) + ti
                qt = work.tile([P, N], f16, name="qt", tag="qt", bufs=1)
                cut = work.tile([P, H], f16, name="cut", tag="cut", bufs=1)
                sut = work.tile([P, H], f16, name="sut", tag="sut", bufs=1)
                nc.sync.dma_start(qt[:], q.ap()[trow * P:(trow + 1) * P, :])
                nc.sync.dma_start(cut[:], cu.ap()[trow * P:(trow + 1) * P, :])
                nc.sync.dma_start(sut[:], su.ap()[trow * P:(trow + 1) * P, :])
                qr = work.tile([P, N], f16, name="qr", tag="qr", bufs=1)
                t1 = work.tile([P, H], f16, name="t1", tag="t1", bufs=1)
                t2 = work.tile([P, H], f16, name="t2", tag="t2", bufs=1)
                qe, qo = qt[:, 0:H], qt[:, H:N]
                nc.vector.tensor_mul(t1[:], qe, cut[:])
                nc.vector.tensor_mul(t2[:], qo, sut[:])
                nc.vector.tensor_sub(qr[:, 0:H], t1[:], t2[:])
                nc.vector.tensor_mul(t1[:], qo, cut[:])
                nc.vector.tensor_mul(t2[:], qe, sut[:])
                nc.vector.tensor_add(qr[:, H:N], t1[:], t2[:])
                for k in range(NCH):
                    pt = ps.tile([P, P], f16, name="tr", tag="tr", bufs=4)
                    nc.tensor.transpose(pt[:], qr[:, k * P:(k + 1) * P], idt[:])
                    nc.scalar.copy(dst[k][:, ti * P:(ti + 1) * P], pt[:])

            def s_block(psrc, row, col, width):
                """Evacuate one accumulated S block [P, width] to s_mat rows
                row.., cols col..; returns the fp16 staging tile."""
                st = work.tile([P, width], f16, name="sst", tag="sst", bufs=3)
                nc.vector.tensor_copy(st[:], psrc[:])
                nc.sync.dma_start(s_mat[row:row + P, col:col + width], st[:])
                return st

            def quad_group(lhs_panel, rhs_panel, m, c0, width):
                """One S block: rows m*P of lhs half, cols [c0, c0+width) of
                rhs half (element offsets)."""
                acc = ps.tile([P, width], f32, name="acc", tag="acc", bufs=4)
                for k in range(NCH):
                    nc.tensor.matmul(
                        acc[:],
                        lhs_panel[k][:, m * P:(m + 1) * P],
                        rhs_panel[k][:, c0:c0 + width],
                        start=(k == 0),
                        stop=(k == NCH - 1),
                    )
                return acc

            def mirror_one(st, sub, r0, c0):
                """Write the transpose of st's sub-block [P, P] (cols sub*P..)
                to s_mat rows r0.., cols c0.. (symmetric fill)."""
                pt = ps.tile([P, P], f16, name="tr", tag="tr", bufs=4)
                nc.tensor.transpose(pt[:], st[:, sub * P:(sub + 1) * P], idt[:])
                ft = work.tile([P, P], f16, name="ft", tag="ft", bufs=3)
                nc.scalar.copy(ft[:], pt[:])
                nc.sync.dma_start(s_mat[r0:r0 + P, c0:c0 + P], ft[:])

            def diag_quadrant_row(pan, q0, m):
                """Row-chunk m of a diagonal quadrant (origin q0 in s_mat):
                compute only blocks on/above the diagonal; mirror-fill the
                strictly-above blocks into the skipped mirror positions."""
                for fc in range(FH):
                    j0 = max(0, m - 4 * fc)
                    if j0 >= F // P:
                        continue
                    width = (F // P - j0) * P
                    c0 = fc * F + j0 * P
                    acc = quad_group(pan, pan, m, c0, width)
                    st = s_block(acc, q0 + m * P, q0 + c0, width)
                    for sub in range(width // P):
                        c = 4 * fc + j0 + sub
                        if c > m:
                            mirror_one(st, sub, q0 + c * P, q0 + m * P)

            def pe_warm(nmm):
                """Junk matmuls (on the const identity, so no data deps) to
                keep the PE activity monitor at full clock while the pipeline
                is otherwise DVE/DMA-bound."""
                wacc = ps.tile([P, P], f32, name="wacc", tag="tr", bufs=4)
                for i in range(nmm):
                    nc.tensor.matmul(
                        wacc[:], idt[:], idt[:],
                        start=True, stop=True, skip_group_check=True,
                    )

            MH = HALF // P   # 8 m-chunks per half
            FH = HALF // F   # 2 f-cols per half

            # ---- build panel A (junk MMs keep the PE clock warm) ----
            pe_warm(48)
            for ti in range(MH):
                build_tile(pa, 0, ti)
                pe_warm(32)

            # ---- S[A,A] (diag-block skipping) interleaved with panel-B build ----
            for i in range(MH):
                diag_quadrant_row(pa, 0, i)
                build_tile(pb, 1, i)

            # ---- S[B,B] (diag-block skipping), S[A,B] (+ mirror to S[B,A]) ----
            pe_warm(16)
            for m in range(MH):
                diag_quadrant_row(pb, HALF, m)
                for fc in range(FH):
                    acc = quad_group(pa, pb, m, fc * F, F)
                    st = s_block(acc, m * P, HALF + fc * F, F)
                    for sub in range(F // P):
                        mirror_one(st, sub, HALF + fc * F + sub * P, m * P)

            # ---- MM2: O = S @ V (S row-panels as lhsT via symmetry) ----
            vts0 = []
            for k in range(NTILES):
                vt = work.tile([P, F], f16, name=f"vt_{k}", tag=f"vt_{k}", bufs=2)
                nc.sync.dma_start(vt[:], v.ap()[k * P:(k + 1) * P, 0:F])
                vts0.append(vt)

            srow = []
            for k in range(NTILES):
                u = panel.tile([P, HALF], f16, name=f"pk_a{2 * k}", tag=f"pk_a{2 * k}")
                w = panel.tile(
                    [P, HALF], f16, name=f"pk_a{2 * k + 1}", tag=f"pk_a{2 * k + 1}"
                )
                nc.sync.dma_start(u[:], s_mat[k * P:(k + 1) * P, 0:HALF])
                nc.sync.dma_start(w[:], s_mat[k * P:(k + 1) * P, HALF:T])
                srow.append((u, w))

            pe_warm(24)
            for j in range(N // F):
                if j == 0:
                    vts = vts0
                else:
                    vts = []
                    for k in range(NTILES):
                        vt = work.tile(
                            [P, F], f16, name=f"vt_{k}", tag=f"vt_{k}", bufs=2
                        )
                        nc.sync.dma_start(
                            vt[:], v.ap()[k * P:(k + 1) * P, j * F:(j + 1) * F]
                        )
                        vts.append(vt)
                for m in range(NTILES):
                    acc = ps.tile([P, F], f32, name="acc", tag="acc", bufs=4)
                    for k in range(NTILES):
                        u, w = srow[k]
                        lhsT = (
                            u[:, m * P:(m + 1) * P]
                            if m < 8
                            else w[:, (m - 8) * P:(m - 7) * P]
                        )
                        nc.tensor.matmul(
                            acc[:], lhsT, vts[k][:],
                            start=(k == 0), stop=(k == NTILES - 1),
                        )
                    ot = work.tile([P, F], f32, name="ot", tag="ot", bufs=3)
                    nc.scalar.copy(ot[:], acc[:])
                    nc.sync.dma_start(
                        o.ap()[m * P:(m + 1) * P, j * F:(j + 1) * F], ot[:]
                    )

    nc.compile()
    return nc


def _tables():
    idx = np.arange(N, dtype=np.float32)
    qq = np.floor(idx / 2.0) * 2.0
    freqs = (1.0 / THETA ** (qq / N) / (2.0 * math.pi)).astype(np.float32)
    fe = freqs[::2]  # [N/2], pairs share a frequency
    ph = (np.arange(T, dtype=np.float32)[:, None] * fe[None, :]).astype(np.float32)
    ang = (np.mod(ph, 1.0) * np.float32(2.0 * math.pi)).astype(np.float32)
    cu_ = (np.cos(ang.astype(np.float64)) / 8.0).astype(np.float16)
    su_ = (np.sin(ang.astype(np.float64)) / 8.0).astype(np.float16)
    return cu_, su_


_NC_CACHE = {}


def kernel(Q, K, V, _trace=False, _tmpdir=None):
    del K  # unused by the reference computation
    if "nc" not in _NC_CACHE:
        _NC_CACHE["nc"] = _build_nc()
    nc = _NC_CACHE["nc"]

    cu_, su_ = _tables()
    ident = np.eye(P, dtype=np.float16)
    Qf = np.asarray(Q, dtype=np.float32)
    # de-interleave feature dim: [evens | odds], fp16
    Qd = np.concatenate([Qf[..., 0::2], Qf[..., 1::2]], axis=-1).astype(np.float16)
    V16 = np.asarray(V, dtype=np.float16)

    in_maps = []
    for c in range(8):
        b, h = divmod(c, NH)
        in_maps.append({
            "q": np.ascontiguousarray(Qd[b, h]),
            "v": np.ascontiguousarray(V16[b, h]),
            "cu": cu_,
            "su": su_,
            "ident": ident,
        })

    kw = {}
    if _trace:
        kw = dict(trace=True, tmpdir=_tmpdir)
    res = run_bass_kernel_spmd(nc, in_maps, list(range(8)), **kw)

    out = np.empty((B, NH, T, N), dtype=np.float32)
    for c in range(8):
        b, h = divmod(c, NH)
        out[b, h] = res.results[c]["o"]
    if _trace:
        kernel.last_exec_time_ns = res.exec_time_ns
    return out



# revision 8
# speedup vs baseline: 1.2557x; 1.2557x over previous
"""Trainium2 Bass kernel for nn_BDHAttention (RoPE(Q) self-score attention, no softmax).

Per (batch, head) slice: QR = rope(Q_s) [T,N]; S = QR @ QR.T / sqrt(N) [T,T];
O = S @ V_s [T,N].  K input is unused by the reference.  B*nh = 8 slices map
1:1 onto the 8 NeuronCores (data/head parallel, no communication).

Device-side structure per core (T=2048, N=4096, P=128):
  - Q arrives fp16, de-interleaved ([evens|odds] along n) AND transposed on
    the host to [N, T], so RoPE runs directly in the transposed layout the
    matmuls need -- no PE transposes for the panels at all.  cos/sin tables
    arrive transposed [N/2, T], pre-scaled by 1/8 each (S picks up 1/64).
  - Build streams in t-quarters (512 cols): DMA Q^T + table slices, RoPE
    in-place on the QR^T tiles (DVE), and MM1 rows unlock progressively.
  - MM1 (fp16): lower-triangle 128-blocks only, row k against cols 0..k as
    soon as t-tile k is built; strictly-lower blocks PE-transposed into the
    mirror position (S symmetric).  S stays RESIDENT in SBUF at full scale:
    s-chunks 0..7 quantized to fp8(e4m3) in DoubleRow pair layout
    [128, 2, 2048], s-chunks 8..15 kept fp16.  Diagonal entries are zeroed
    in the fp8 panels; exact diagonals are captured per-partition and
    re-applied via tiny diag-matrix matmuls in MM2.
  - MM2: O = S @ V with a mixed-precision contraction: 4 fp8 DoubleRow
    pair-matmuls (2x PE rate) + 8 fp16 matmuls + 1 diag fp16 matmul per
    output tile, all accumulating in one fp32 PSUM bank.  V streamed fp16,
    low s-chunks quantized to fp8 on device.  O written fp16, host casts
    to fp32.  Empirical rel-err of this scheme ~1.7e-2 (gate 2e-2).
"""

import math
import sys

sys.path.insert(0, "/opt/trn_rl_repo")

import numpy as np

import concourse.bacc as bacc
import concourse.mybir as mybir
import concourse.tile as tile
from concourse.bass_utils import run_bass_kernel_spmd

B, NH, T, N = 2, 4, 2048, 4096
THETA = 2 ** 16
P = 128
NCH = N // P             # 32 n-chunks
NT = T // P              # 16 t-tiles
QUART = 512              # t-cols per build quarter (4 t-tiles)
NQ = T // QUART          # 4 quarters
K8 = 8                   # s-chunks 0..K8-1 are fp8 in MM2; K8..15 fp16
F = 512                  # MM2 j-column width (one fp32 PSUM bank)

f8 = mybir.dt.float8e4
f16 = mybir.dt.float16
f32 = mybir.dt.float32
MULT = mybir.AluOpType.mult
AXX = mybir.AxisListType.X
DROW = mybir.MatmulPerfMode.DoubleRow


def _build_nc():
    nc = bacc.Bacc("TRN2", target_bir_lowering=False, debug=False, num_devices=8)

    qT = nc.dram_tensor("qt", [N, T], f16, kind="ExternalInput")
    cuT = nc.dram_tensor("cu", [N // 2, T], f16, kind="ExternalInput")
    suT = nc.dram_tensor("su", [N // 2, T], f16, kind="ExternalInput")
    v = nc.dram_tensor("v", [T, N], f16, kind="ExternalInput")
    ident = nc.dram_tensor("ident", [P, P], f16, kind="ExternalInput")
    o = nc.dram_tensor("o", [T, N], f16, kind="ExternalOutput")

    with tile.TileContext(nc) as tc:
        with (
            tc.tile_pool(name="const", bufs=1) as const,
            tc.tile_pool(name="s16p", bufs=1) as s16p,
            tc.tile_pool(name="s8p", bufs=1) as s8p,
            tc.tile_pool(name="dtp", bufs=1) as dtp,
            tc.tile_pool(name="ps", bufs=1, space="PSUM") as ps,
            tc.tile_pool(name="work", bufs=1) as work,
        ):
            idt = const.tile([P, P], f16, name="idt")
            nc.sync.dma_start(idt[:], ident.ap())
            wsrc = const.tile([P, F], f16, name="wsrc")
            nc.vector.memset(wsrc[:], 0.125)

            qrp = tc.alloc_tile_pool(name="qrp", bufs=1)
            # persistent panels
            qr_t = [
                [
                    qrp.tile([P, QUART], f16, name=f"qr{q}_{c}", tag=f"qr{q}_{c}")
                    for c in range(NCH)
                ]
                for q in range(NQ)
            ]
            s16 = [
                s16p.tile([P, T], f16, name=f"s16_{k}", tag=f"s16_{k}")
                for k in range(NT - K8)
            ]
            sp8 = [
                s8p.tile([P, 2, T], f8, name=f"sp8_{i}", tag=f"sp8_{i}")
                for i in range(K8 // 2)
            ]
            dti = [
                dtp.tile([P, P], f16, name=f"dti{m}", tag=f"dti{m}")
                for m in range(K8)
            ]

            # PE warmup: keep the clock ramping while the first quarter streams in
            for _ in range(56):
                wacc = ps.tile([P, F], f32, name="wacc", tag="tr", bufs=2)
                nc.tensor.matmul(
                    wacc[:], idt[:], wsrc[:],
                    start=True, stop=True, skip_group_check=True,
                )

            pending = []  # deferred mirror jobs: (src_ap, c, k)

            def flush_mirrors():
                for src_ap, c, k in pending:
                    pt = ps.tile([P, P], f16, name="pt", tag="tr", bufs=2)
                    nc.tensor.transpose(pt[:], src_ap, idt[:])
                    if c < K8:
                        dst = sp8[c // 2][:, (c % 2):(c % 2) + 1, k * P:(k + 1) * P]
                        nc.vector.tensor_copy(dst, pt[:])
                    else:
                        nc.scalar.copy(s16[c - K8][:, k * P:(k + 1) * P], pt[:])
                pending.clear()

            def emit_row(k):
                """MM1 row k: blocks (k, c) for c <= k, evac + queue mirrors."""
                ngroups = (k + 4) // 4
                for g in range(ngroups):
                    c_lo = g * 4
                    ntile = min(k + 1 - c_lo, 4)
                    width = ntile * P
                    acc = ps.tile([P, F], f32, name="acc", tag="acc", bufs=3)
                    for cc in range(NCH):
                        nc.tensor.matmul(
                            acc[:, :width],
                            qr_t[k // 4][cc][:, (k % 4) * P:(k % 4 + 1) * P],
                            qr_t[g][cc][:, :width],
                            start=(cc == 0),
                            stop=(cc == NCH - 1),
                        )
                    if k >= K8:
                        dst = s16[k - K8][:, c_lo * P:c_lo * P + width]
                        nc.vector.tensor_copy(dst, acc[:, :width])
                        srcs = [
                            s16[k - K8][:, (c_lo + sub) * P:(c_lo + sub + 1) * P]
                            for sub in range(ntile)
                        ]
                    else:
                        st = work.tile([P, F], f16, name="st", tag="st", bufs=2)
                        nc.vector.tensor_copy(st[:, :width], acc[:, :width])
                        for sub in range(ntile):
                            c = c_lo + sub
                            dst8 = sp8[k // 2][:, (k % 2):(k % 2) + 1, c * P:(c + 1) * P]
                            if c == k:
                                # exact diag -> dti[k] (full scale), zero it in fp8
                                dg = work.tile([P, P], f32, name="dg", tag="dg", bufs=2)
                                nc.vector.tensor_mul(
                                    dg[:], acc[:, sub * P:(sub + 1) * P], idt[:]
                                )
                                dv = dtp.tile([P, 1], f32, name=f"dv{k}", tag=f"dv{k}")
                                nc.vector.tensor_reduce(dv[:], dg[:], axis=AXX, op=mybir.AluOpType.add)
                                nc.vector.tensor_scalar(
                                    dti[k][:], idt[:], dv[:], None, op0=MULT
                                )
                                nc.vector.tensor_sub(
                                    dst8, st[:, sub * P:(sub + 1) * P], dti[k][:]
                                )
                            else:
                                nc.vector.tensor_copy(dst8, st[:, sub * P:(sub + 1) * P])
                        srcs = [
                            st[:, sub * P:(sub + 1) * P] for sub in range(ntile)
                        ]
                    flush_mirrors()
                    pending.extend(
                        (srcs[sub], c_lo + sub, k)
                        for sub in range(ntile)
                        if c_lo + sub < k
                    )

            # ---- phase 1: streamed build + progressive MM1 ----
            with tc.tile_pool(name="tab", bufs=1) as tab:
                for q in range(NQ):
                    cols = slice(q * QUART, (q + 1) * QUART)
                    for i in range(NCH // 2):
                        cut = tab.tile([P, QUART], f16, name="cut", tag=f"cu{i % 3}", bufs=2)
                        sut = tab.tile([P, QUART], f16, name="sut", tag=f"su{i % 3}", bufs=2)
                        nc.sync.dma_start(cut[:], cuT.ap()[i * P:(i + 1) * P, cols])
                        nc.sync.dma_start(sut[:], suT.ap()[i * P:(i + 1) * P, cols])
                        qe_t, qo_t = qr_t[q][i], qr_t[q][i + 16]
                        nc.sync.dma_start(qe_t[:], qT.ap()[i * P:(i + 1) * P, cols])
                        nc.sync.dma_start(
                            qo_t[:], qT.ap()[(i + 16) * P:(i + 17) * P, cols]
                        )
                        t1 = work.tile([P, QUART], f16, name="t1", tag="t1", bufs=2)
                        t2 = work.tile([P, QUART], f16, name="t2", tag="t2", bufs=2)
                        t3 = work.tile([P, QUART], f16, name="t3", tag="t3", bufs=2)
                        t4 = work.tile([P, QUART], f16, name="t4", tag="t4", bufs=2)
                        nc.vector.tensor_mul(t1[:], qe_t[:], cut[:])
                        nc.vector.tensor_mul(t2[:], qo_t[:], sut[:])
                        nc.vector.tensor_mul(t3[:], qo_t[:], cut[:])
                        nc.vector.tensor_mul(t4[:], qe_t[:], sut[:])
                        nc.vector.tensor_sub(qe_t[:], t1[:], t2[:])
                        nc.vector.tensor_add(qo_t[:], t3[:], t4[:])
                    for kt in range(4):
                        emit_row(q * 4 + kt)
            flush_mirrors()
            qrp.release()

            # ---- phase 2: MM2, O = S @ V (mixed fp8-DoubleRow / fp16) ----
            with tc.tile_pool(name="vst", bufs=1) as vst:
                for j in range(N // F):
                    jcols = slice(j * F, (j + 1) * F)
                    v16 = []
                    for kk in range(NT):
                        vt = vst.tile([P, F], f16, name=f"v{kk}", tag=f"v{kk}", bufs=2)
                        nc.sync.dma_start(vt[:], v.ap()[kk * P:(kk + 1) * P, jcols])
                        v16.append(vt)
                    v8 = []
                    for i in range(K8 // 2):
                        p8 = vst.tile([P, 2, F], f8, name=f"v8_{i}", tag=f"v8_{i}", bufs=2)
                        nc.vector.tensor_copy(p8[:, 0:1, :], v16[2 * i][:])
                        nc.vector.tensor_copy(p8[:, 1:2, :], v16[2 * i + 1][:])
                        v8.append(p8)
                    for m in range(NT):
                        macc = ps.tile([P, F], f32, name="macc", tag="macc", bufs=3)
                        for i in range(K8 // 2):
                            nc.tensor.matmul(
                                macc[:],
                                sp8[i][:, :, m * P:(m + 1) * P],
                                v8[i][:],
                                start=(i == 0),
                                stop=False,
                                perf_mode=DROW,
                            )
                        for kk in range(NT - K8):
                            nc.tensor.matmul(
                                macc[:],
                                s16[kk][:, m * P:(m + 1) * P],
                                v16[K8 + kk][:],
                                start=False,
                                stop=(kk == NT - K8 - 1 and m >= K8),
                            )
                        if m < K8:
                            nc.tensor.matmul(
                                macc[:], dti[m][:], v16[m][:], start=False, stop=True
                            )
                        ot = work.tile([P, F], f16, name="ot", tag="ot", bufs=3)
                        nc.vector.tensor_copy(ot[:], macc[:])
                        nc.sync.dma_start(o.ap()[m * P:(m + 1) * P, jcols], ot[:])

    nc.compile()
    return nc


def _tables():
    idx = np.arange(N, dtype=np.float32)
    qq = np.floor(idx / 2.0) * 2.0
    freqs = (1.0 / THETA ** (qq / N) / (2.0 * math.pi)).astype(np.float32)
    fe = freqs[::2]  # [N/2], pairs share a frequency
    ph = (np.arange(T, dtype=np.float32)[:, None] * fe[None, :]).astype(np.float32)
    ang = (np.mod(ph, 1.0) * np.float32(2.0 * math.pi)).astype(np.float32)
    cu_ = (np.cos(ang.astype(np.float64)) / 8.0).astype(np.float16)
    su_ = (np.sin(ang.astype(np.float64)) / 8.0).astype(np.float16)
    return np.ascontiguousarray(cu_.T), np.ascontiguousarray(su_.T)


_NC_CACHE = {}


def kernel(Q, K, V, _trace=False, _tmpdir=None):
    del K  # unused by the reference computation
    if "nc" not in _NC_CACHE:
        _NC_CACHE["nc"] = _build_nc()
    nc = _NC_CACHE["nc"]

    cuT, suT = _tables()
    ident = np.eye(P, dtype=np.float16)
    Qf = np.asarray(Q, dtype=np.float32)
    # de-interleave feature dim ([evens|odds]; harmless: it permutes the
    # contraction dim shared by both MM1 operands), fp16, transpose to [N, T]
    Qd = np.concatenate([Qf[..., 0::2], Qf[..., 1::2]], axis=-1).astype(np.float16)
    V16 = np.asarray(V, dtype=np.float16)

    in_maps = []
    for c in range(8):
        b, h = divmod(c, NH)
        in_maps.append({
            "qt": np.ascontiguousarray(Qd[b, h].T),
            "cu": cuT,
            "su": suT,
            "v": np.ascontiguousarray(V16[b, h]),
            "ident": ident,
        })

    kw = {}
    if _trace:
        kw = dict(trace=True, tmpdir=_tmpdir)
    res = run_bass_kernel_spmd(nc, in_maps, list(range(8)), **kw)

    out = np.empty((B, NH, T, N), dtype=np.float32)
    for c in range(8):
        b, h = divmod(c, NH)
        out[b, h] = np.asarray(res.results[c]["o"]).astype(np.float32)
    if _trace:
        kernel.last_exec_time_ns = res.exec_time_ns
    return out
